# revision 21
# baseline (speedup 1.0000x reference)
"""Trainium2 Bass kernel for ARC transformer encoder layer (attention + ConvGLU MLP).

Sharding: data-parallel over batch, one batch element per NeuronCore (8 cores).
On-device layout is feature-major ([features, seq]) everywhere, which makes every
matmul contraction land on the partition dim with zero on-device transposes.
Softmax is computed in "transposed" orientation (keys on partitions) with the
denominator obtained from an extra ones-column appended to V; normalization uses
a DRAM-bounce partition-broadcast of the reciprocal row.

Host-side prep: transpose x per batch, fold g1/g2 and the attention scale into
the weights, precompute RoPE cos/sin tables (rotate-half folded into a shifted
"sin2" table so the on-device rotation is a plain partition-shift done by DMA).
"""

from contextlib import ExitStack

import numpy as np
import ml_dtypes

import concourse.bass as bass
import concourse.tile as tile
import concourse.mybir as mybir
from concourse.bass_utils import run_bass_kernel_spmd

dt = mybir.dt
AF = mybir.ActivationFunctionType
ALU = mybir.AluOpType

B, S, E = 8, 1025, 1024
NH, HD = 16, 64
HID = 2730
EPS = 1e-6
SCALE = HD ** -0.5
NCORES = 8

P = 128
ET = E // P            # 8 e-tiles
KT = 9                 # key tiles: 8 full + 1 single row
CT = 22                # fc hidden channel tiles (21*128 + 42)
CHUNKS = [(0, 512), (512, 512), (1024, 1)]           # seq chunks
QKV_CHUNKS = [(0, 512), (512, 512), (1023, 2)]       # even tail width (f32r needs even N)
IMG_CHUNKS = [(1, 512), (513, 512), (0, 2)]          # image-row-aligned (task last, even width for f32r)
HT_LAST = HID - 21 * P                               # 42


def _ct_rows(ct):
    return P if ct < CT - 1 else HT_LAST


def _kt_rows(kt):
    return P if kt < KT - 1 else 1


def build_kernel(VAUG_MEMSET=True, DEBUG=False):
    nc = bass.Bass()

    # ---------------- DRAM I/O ----------------
    xT = nc.dram_tensor("xT", [E, S], dt.float32, kind="ExternalInput")
    wqk = nc.dram_tensor("wqk", [16, P, ET * P], dt.bfloat16, kind="ExternalInput")
    bqk = nc.dram_tensor("bqk", [P, 16], dt.float32, kind="ExternalInput")
    wv = nc.dram_tensor("wv", [E, E], dt.bfloat16, kind="ExternalInput")
    vaug_init = nc.dram_tensor("vaug_init", [P, NH * (HD + 1)], dt.bfloat16, kind="ExternalInput")
    wproj = nc.dram_tensor("wproj", [ET, P, E], dt.bfloat16, kind="ExternalInput")
    bproj = nc.dram_tensor("bproj", [P, ET], dt.float32, kind="ExternalInput")
    cosb = nc.dram_tensor("cosb", [P, S], dt.float32, kind="ExternalInput")
    sin2b = nc.dram_tensor("sin2b", [P, S], dt.float32, kind="ExternalInput")
    wxg = nc.dram_tensor("wxg", [CT, P, ET * P], dt.float32r, kind="ExternalInput")
    bxg = nc.dram_tensor("bxg", [CT, P, 1], dt.float32, kind="ExternalInput")
    wvg = nc.dram_tensor("wvg", [CT, P, ET * P], dt.float32r, kind="ExternalInput")
    bvg = nc.dram_tensor("bvg", [CT, P, 1], dt.float32, kind="ExternalInput")
    dwt = nc.dram_tensor("dwt", [CT, P, 9], dt.float32, kind="ExternalInput")
    dwb = nc.dram_tensor("dwb", [CT, P, 1], dt.float32, kind="ExternalInput")
    wfc2 = nc.dram_tensor("wfc2", [ET, P, CT * P], dt.bfloat16, kind="ExternalInput")
    bfc2 = nc.dram_tensor("bfc2", [P, ET], dt.float32, kind="ExternalInput")
    yT = nc.dram_tensor("yT", [E, S], dt.float32, kind="ExternalOutput")

    if DEBUG:
        dbg_qk = nc.dram_tensor("dbg_qk", [16, P, S], dt.float32, kind="ExternalOutput")
        dbg_v = nc.dram_tensor("dbg_v", [KT, P, NH * (HD + 1)], dt.float32, kind="ExternalOutput")
        dbg_of = nc.dram_tensor("dbg_of", [ET, P, S], dt.float32, kind="ExternalOutput")
        dbg_x2 = nc.dram_tensor("dbg_x2", [ET, P, S], dt.float32, kind="ExternalOutput")
        dbg_h = nc.dram_tensor("dbg_h", [CT, P, S], dt.float32, kind="ExternalOutput")
    scr_rn1 = nc.dram_tensor("scr_rn1", [S], dt.float32)
    scr_rn2 = nc.dram_tensor("scr_rn2", [S], dt.float32)
    scr_den = nc.dram_tensor("scr_den", [NH, S], dt.float32)

    with tile.TileContext(nc) as tc, ExitStack() as top:
        # ---------------- persistent pools ----------------
        const_pool = top.enter_context(tc.tile_pool(name="consts", bufs=1))
        x_pool = top.enter_context(tc.tile_pool(name="x", bufs=1))

        ones_r = const_pool.tile([P, 1], dt.bfloat16, name="ones_r")
        nc.vector.memset(ones_r[:], 1.0)
        bqk_t = const_pool.tile([P, 16], dt.float32, name="bqk_t")
        nc.sync.dma_start(bqk_t[:], bqk[:, :])
        bproj_t = const_pool.tile([P, ET], dt.float32, name="bproj_t")
        nc.sync.dma_start(bproj_t[:], bproj[:, :])
        bfc2_t = const_pool.tile([P, ET], dt.float32, name="bfc2_t")
        nc.sync.dma_start(bfc2_t[:], bfc2[:, :])
        eps_t = const_pool.tile([P, 1], dt.float32, name="eps_t")
        nc.vector.memset(eps_t[:], EPS)

        x_t = [x_pool.tile([P, S], dt.float32, name=f"x_{t}") for t in range(ET)]
        for t in range(ET):
            nc.sync.dma_start(x_t[t][:], xT[t * P:(t + 1) * P, :])

        # =========== helper: rmsnorm (feature-major) ===========
        def rmsnorm(src_tiles, scr, xn_pool, xn_dtype, ph):
            with tc.tile_pool(name=f"rms{ph}", bufs=3) as sp, \
                 tc.tile_pool(name=f"rmsp{ph}", bufs=1, space="PSUM") as pp:
                ss = pp.tile([1, S], dt.float32, name=f"ss{ph}")
                for t in range(ET):
                    sq = sp.tile([P, S], dt.bfloat16, name=f"sq{ph}", tag=f"sq{ph}")
                    nc.scalar.activation(sq[:], src_tiles[t][:], AF.Square)
                    for (off, w) in CHUNKS:
                        nc.tensor.matmul(ss[:, off:off + w], ones_r[:], sq[:, off:off + w],
                                         start=(t == 0), stop=(t == ET - 1))
                rn = sp.tile([1, S], dt.float32, name=f"rn{ph}", tag=f"rn{ph}")
                rnb = sp.tile([P, S], dt.float32, name=f"rnb{ph}", tag=f"rnb{ph}")
                for (off, w) in CHUNKS:
                    nc.scalar.activation(rn[0:1, off:off + w], ss[0:1, off:off + w],
                                         AF.Sqrt, scale=1.0 / E, bias=eps_t[0:1, :])
                    nc.vector.reciprocal(rn[0:1, off:off + w], rn[0:1, off:off + w])
                    nc.sync.dma_start(scr[off:off + w], rn[0:1, off:off + w])
                    nc.sync.dma_start(rnb[:, off:off + w],
                                      scr[off:off + w][None, :].broadcast_to([P, w]))
                xn_t = [xn_pool.tile([P, S], xn_dtype, name=f"xn{ph}_{t}") for t in range(ET)]
                for (off, w) in CHUNKS:
                    for t in range(ET):
                        nc.vector.tensor_mul(xn_t[t][:, off:off + w], src_tiles[t][:, off:off + w],
                                             rnb[:, off:off + w])
            return xn_t

        # ============ attention block (phases 1-4) ============
        with tc.tile_pool(name="qk", bufs=1) as qk_pool, \
             tc.tile_pool(name="vaug", bufs=1) as vaug_pool:
            qk_t = [qk_pool.tile([P, S], dt.bfloat16, name=f"qk_{m}") for m in range(16)]
            v_t = [vaug_pool.tile([P, NH * (HD + 1)], dt.bfloat16, name=f"v_{st}") for st in range(KT)]

            with tc.tile_pool(name="xn1p", bufs=1) as xn1_pool:
                xn_t = rmsnorm(x_t, scr_rn1, xn1_pool, dt.bfloat16, 1)

                # ---------- qk matmuls + rope ----------
                with tc.tile_pool(name="ropec", bufs=1) as rcp, \
                     tc.tile_pool(name="wq_pool", bufs=3) as wqp, \
                     tc.tile_pool(name="rope_pool", bufs=3) as rp, \
                     tc.tile_pool(name="qkv_psum", bufs=3, space="PSUM") as qps:
                    cos_t = rcp.tile([P, S], dt.float32, name="cos_t")
                    nc.sync.dma_start(cos_t[:], cosb[:, :])
                    sin2_t = rcp.tile([P, S], dt.float32, name="sin2_t")
                    nc.sync.dma_start(sin2_t[:], sin2b[:, :])

                    for m in range(16):
                        wt = wqp.tile([P, ET * P], dt.bfloat16, name="wt", tag="wqk")
                        nc.sync.dma_start(wt[:], wqk[m])
                        for ci, (off, w) in enumerate(QKV_CHUNKS):
                            ps = qps.tile([P, 512], dt.float32, name="qkps", tag="qkps")
                            for e in range(ET):
                                nc.tensor.matmul(ps[:, :w], wt[:, e * P:(e + 1) * P],
                                                 xn_t[e][:, off:off + w],
                                                 start=(e == 0), stop=(e == ET - 1))
                            # rope: out = (ps+b)*cos + shift32((ps+b)*sin2)
                            ev = rp.tile([P, 512], dt.bfloat16, name="ev", tag="ev")
                            nc.scalar.activation(ev[:, :w], ps[:, :w], AF.Identity, bias=bqk_t[:, m:m + 1])
                            t1 = rp.tile([P, 512], dt.bfloat16, name="t1", tag="t1")
                            nc.gpsimd.tensor_mul(t1[:, :w], ev[:, :w], sin2_t[:, off:off + w])
                            tc_ = rp.tile([P, 512], dt.bfloat16, name="tc_", tag="tc_")
                            nc.vector.tensor_mul(tc_[:, :w], ev[:, :w], cos_t[:, off:off + w])
                            rot = rp.tile([P, 512], dt.bfloat16, name="rot", tag="rot")
                            eng = [nc.scalar, nc.sync][ci % 2]
                            eng.dma_start(rot[0:32, :w], t1[32:64, :w])
                            eng.dma_start(rot[32:64, :w], t1[0:32, :w])
                            eng.dma_start(rot[64:96, :w], t1[96:128, :w])
                            eng.dma_start(rot[96:128, :w], t1[64:96, :w])
                            nc.vector.tensor_add(qk_t[m][:, off:off + w], tc_[:, :w], rot[:, :w])

                    # ---------- v (token-major, augmented ones col) ----------
                    for st in range(KT):
                        if VAUG_MEMSET:
                            nc.vector.memset(v_t[st][:], 0.0)
                            nc.gpsimd.memset(v_t[st].rearrange("p (h d) -> p h d", d=HD + 1)[:, :, HD:HD + 1], 1.0)
                        else:
                            nc.sync.dma_start(v_t[st][:], vaug_init[:, :])
                    with tc.tile_pool(name="wv_pool", bufs=3) as wvp, \
                         tc.tile_pool(name="v_psum", bufs=5, space="PSUM") as vps_pool:
                        for blk in ((0, 1, 2, 3, 4), (5, 6, 7, 8)):
                            for fc in range(2):
                                pss = {st: vps_pool.tile([P, 512], dt.float32, name=f"vps{st}", tag="vps")
                                       for st in blk}
                                for e in range(ET):
                                    wvt = wvp.tile([P, 512], dt.bfloat16, name="wvt", tag="wv")
                                    nc.sync.dma_start(wvt[:], wv[e * P:(e + 1) * P, fc * 512:(fc + 1) * 512])
                                    for st in blk:
                                        rows = _kt_rows(st)
                                        nc.tensor.matmul(pss[st][:rows, :],
                                                         xn_t[e][:, st * P:st * P + rows],
                                                         wvt[:], start=(e == 0), stop=(e == ET - 1))
                                for st in blk:
                                    rows = _kt_rows(st)
                                    dst = v_t[st].rearrange("p (h d) -> p h d", d=HD + 1)[:rows, fc * 8:(fc + 1) * 8, 0:HD]
                                    srcp = pss[st].rearrange("p (h d) -> p h d", d=HD)[:rows]
                                    nc.vector.tensor_add(dst, srcp, dst)

            if DEBUG:
                for m in range(16):
                    nc.gpsimd.dma_start(dbg_qk[m], qk_t[m][:])
                for st in range(KT):
                    nc.gpsimd.dma_start(dbg_v[st], v_t[st][:])
            # ---------------- attention + proj ----------------
            with tc.tile_pool(name="of", bufs=1) as of_pool:
                out_f = [of_pool.tile([P, S], dt.bfloat16, name=f"of_{t}") for t in range(ET)]

                with tc.tile_pool(name="exp_pool", bufs=10) as ep, \
                     tc.tile_pool(name="den_pool", bufs=4) as dp, \
                     tc.tile_pool(name="sc_psum", bufs=2, space="PSUM") as scp, \
                     tc.tile_pool(name="ov_psum", bufs=4, space="PSUM") as ovp:
                    NG = 5
                    for (off, w) in [CHUNKS[2], CHUNKS[0], CHUNKS[1]]:
                        for h in range(NH):
                            qt = qk_t[h // 2]
                            kt_ = qk_t[8 + h // 2]
                            hb = 64 * (h % 2)
                            q_sl = qt[hb:hb + 64, off:off + w]
                            exp_tiles = []
                            for g in range(NG):
                                ps = scp.tile([P, 2 * 512], dt.float32, name="scps", tag="scps")
                                ex = ep.tile([P, 2 * 512], dt.bfloat16, name="ex", tag="ex")
                                n_in_g = 2 if g < NG - 1 else 1
                                for j in range(n_in_g):
                                    kti = 2 * g + j
                                    rows = _kt_rows(kti)
                                    nc.tensor.matmul(ps[:rows, j * 512:j * 512 + w],
                                                     kt_[hb:hb + 64, kti * P:kti * P + rows],
                                                     q_sl, start=True, stop=True)
                                if w == 512:
                                    if g < NG - 1:
                                        nc.scalar.activation(ex[:], ps[:], AF.Exp)
                                    else:
                                        nc.scalar.activation(ex[0:1, 0:512], ps[0:1, 0:512], AF.Exp)
                                else:
                                    for j in range(n_in_g):
                                        rows = _kt_rows(2 * g + j)
                                        nc.scalar.activation(ex[:rows, j * 512:j * 512 + w],
                                                             ps[:rows, j * 512:j * 512 + w], AF.Exp)
                                exp_tiles.append(ex)
                            po = ovp.tile([HD + 1, 512], dt.float32, name="po", tag="po")
                            for kti in range(KT):
                                rows = _kt_rows(kti)
                                vsl = v_t[kti].rearrange("p (h d) -> p h d", d=HD + 1)[:rows, h]
                                nc.tensor.matmul(po[:, :w], vsl,
                                                 exp_tiles[kti // 2][:rows, (kti % 2) * 512:(kti % 2) * 512 + w],
                                                 start=(kti == 0), stop=(kti == KT - 1))
                            den = dp.tile([P, 512], dt.float32, name="den", tag="den")
                            nc.vector.reciprocal(den[64:65, :w], po[64:65, :w])
                            nc.sync.dma_start(scr_den[h, off:off + w], den[64:65, :w])
                            nc.sync.dma_start(den[0:64, :w],
                                               scr_den[h, off:off + w][None, :].broadcast_to([64, w]))
                            nc.vector.tensor_mul(out_f[h // 2][hb:hb + 64, off:off + w],
                                                 po[0:64, :w], den[0:64, :w])

                # ---------------- proj + residual ----------------
                with tc.tile_pool(name="wp_pool", bufs=3) as wpp, \
                     tc.tile_pool(name="pj_psum", bufs=4, space="PSUM") as pjp:
                    for m in range(ET):
                        wt = wpp.tile([P, ET * P], dt.bfloat16, name="wpt", tag="wp")
                        nc.scalar.dma_start(wt[:], wproj[m])
                        for (off, w) in CHUNKS:
                            ps = pjp.tile([P, 512], dt.float32, name="pjps", tag="pjps")
                            for e in range(ET):
                                nc.tensor.matmul(ps[:, :w], wt[:, e * P:(e + 1) * P],
                                                 out_f[e][:, off:off + w],
                                                 start=(e == 0), stop=(e == ET - 1))
                            nc.vector.scalar_tensor_tensor(x_t[m][:, off:off + w], ps[:, :w],
                                                           bproj_t[:, m:m + 1], x_t[m][:, off:off + w],
                                                           op0=ALU.add, op1=ALU.add)

        if DEBUG:
            for t in range(ET):
                nc.sync.dma_start(dbg_x2[t], x_t[t][:])
        # ================= ConvGLU MLP block =================
        with tc.tile_pool(name="ht", bufs=1) as ht_pool:
            h_t = [ht_pool.tile([P, S], dt.bfloat16, name=f"h_{ct}") for ct in range(CT)]

            with tc.tile_pool(name="xn2p", bufs=1) as xn2_pool:
                xn2_t = rmsnorm(x_t, scr_rn2, xn2_pool, dt.float32r, 2)

                with tc.tile_pool(name="wg_pool", bufs=4) as wgp, \
                     tc.tile_pool(name="cg_pool", bufs=4) as cgp, \
                     tc.tile_pool(name="padp", bufs=1) as padp, \
                     tc.tile_pool(name="fc1_psum", bufs=6, space="PSUM") as fp1:
                    pads = [padp.tile([P, 34 * 34], dt.bfloat16, name=f"pad{i}") for i in range(3)]
                    for i in range(3):
                        nc.vector.memset(pads[i][:], 0.0)
                    TAP_OFF = [(dy, dx) for dy in (-1, 0, 1) for dx in (-1, 0, 1)]
                    for ct in range(CT):
                        rows = _ct_rows(ct)
                        wt = wgp.tile([P, ET * P], dt.float32r, name="wxt", tag="wg")
                        nc.sync.dma_start(wt[:], wxg[ct])
                        dwtt = cgp.tile([P, 9], dt.float32, name="dwtt", tag="dwtt")
                        nc.sync.dma_start(dwtt[:rows, :], dwt[ct, 0:rows, :])
                        dwbt = cgp.tile([P, 1], dt.float32, name="dwbt", tag="dwbt")
                        nc.sync.dma_start(dwbt[:rows, :], dwb[ct, 0:rows, :])
                        bxt = cgp.tile([P, 1], dt.float32, name="bxt", tag="bxt")
                        nc.sync.dma_start(bxt[:rows, :], bxg[ct, 0:rows, :])
                        bvt = cgp.tile([P, 1], dt.float32, name="bvt", tag="bvt")
                        nc.sync.dma_start(bvt[:rows, :], bvg[ct, 0:rows, :])

                        pad = pads[ct % 3]
                        xca = cgp.tile([P, S], dt.bfloat16, name="xca", tag="xca")
                        xcb = cgp.tile([P, S], dt.bfloat16, name="xcb", tag="xcb")
                        for ci, (off, w) in enumerate(IMG_CHUNKS):
                            ps = fp1.tile([P, 512], dt.float32, name="f1ps", tag="f1ps")
                            for e in range(ET):
                                nc.tensor.matmul(ps[:rows, :w], wt[:, e * P:e * P + rows],
                                                 xn2_t[e][:, off:off + w],
                                                 start=(e == 0), stop=(e == ET - 1))
                            if ci < 2:
                                dst = pad.rearrange("p (y x) -> p y x", x=34)[:rows, 1 + ci * 16:1 + (ci + 1) * 16, 1:33]
                                nc.scalar.activation(dst, ps.rearrange("p (y x) -> p y x", x=32)[:rows],
                                                     AF.Identity, bias=bxt[:rows])
                            else:
                                nc.scalar.activation(xcb[:rows, 0:2], ps[:rows, 0:2],
                                                     AF.Identity, bias=bxt[:rows])
                        wt2 = wgp.tile([P, ET * P], dt.float32r, name="wvt2", tag="wg")
                        nc.scalar.dma_start(wt2[:], wvg[ct])
                        vg = cgp.tile([P, S], dt.bfloat16, name="vg", tag="vg")
                        for ci, (off, w) in enumerate(IMG_CHUNKS):
                            ps = fp1.tile([P, 512], dt.float32, name="f1ps2", tag="f1ps")
                            for e in range(ET):
                                nc.tensor.matmul(ps[:rows, :w], wt2[:, e * P:e * P + rows],
                                                 xn2_t[e][:, off:off + w],
                                                 start=(e == 0), stop=(e == ET - 1))
                            nc.scalar.activation(vg[:rows, off:off + w], ps[:rows, :w],
                                                 AF.Identity, bias=bvt[:rows])
                        # depthwise 3x3: taps 0-5 on DVE into xca, taps 6-8 on Pool into xcb
                        xai = xca[:rows, 1:S].rearrange("p (y x) -> p y x", x=32)
                        xbi = xcb[:rows, 1:S].rearrange("p (y x) -> p y x", x=32)
                        padi = pad.rearrange("p (y x) -> p y x", x=34)
                        tmp8 = cgp.tile([P, S], dt.bfloat16, name="tmp8", tag="tmp8")
                        t8i = tmp8[:rows, 1:S].rearrange("p (y x) -> p y x", x=32)
                        for ti, (dy, dx) in enumerate(TAP_OFF):
                            srcp = padi[:rows, 1 + dy:33 + dy, 1 + dx:33 + dx]
                            if ti == 0:
                                nc.vector.tensor_scalar_mul(xai, srcp, dwtt[:rows, ti:ti + 1])
                            elif ti < 7:
                                nc.vector.scalar_tensor_tensor(xai, srcp, dwtt[:rows, ti:ti + 1],
                                                               xai, op0=ALU.mult, op1=ALU.add)
                            elif ti == 7:
                                nc.gpsimd.tensor_scalar_mul(xbi, srcp, dwtt[:rows, ti:ti + 1])
                            else:
                                nc.gpsimd.tensor_scalar_mul(t8i, srcp, dwtt[:rows, ti:ti + 1])
                        nc.gpsimd.tensor_add(xbi, t8i, xbi)
                        nc.gpsimd.tensor_add(xbi, xai, xbi)
                        gl = cgp.tile([P, S], dt.bfloat16, name="gl", tag="gl")
                        nc.scalar.activation(gl[:rows, 1:S], xcb[:rows, 1:S], AF.Gelu, bias=dwbt[:rows])
                        nc.scalar.activation(gl[:rows, 0:1], xcb[:rows, 0:1], AF.Gelu)
                        nc.vector.tensor_mul(h_t[ct][:rows, :], gl[:rows, :], vg[:rows, :])

            if DEBUG:
                for ct in range(CT):
                    nc.gpsimd.dma_start(dbg_h[ct], h_t[ct][:])
            # ================= fc2 + residual =================
            with tc.tile_pool(name="w2_pool", bufs=3) as w2p, \
                 tc.tile_pool(name="y_pool", bufs=3) as yp, \
                 tc.tile_pool(name="fc2_psum", bufs=4, space="PSUM") as fp2:
                for m in range(ET):
                    wt = w2p.tile([P, CT * P], dt.bfloat16, name="w2t", tag="w2")
                    nc.sync.dma_start(wt[:], wfc2[m])
                    yt = yp.tile([P, S], dt.float32, name="yt", tag="yt")
                    for (off, w) in CHUNKS:
                        ps = fp2.tile([P, 512], dt.float32, name="f2ps", tag="f2ps")
                        for kt_i in range(CT):
                            rows = _ct_rows(kt_i)
                            nc.tensor.matmul(ps[:, :w], wt[0:rows, kt_i * P:(kt_i + 1) * P],
                                             h_t[kt_i][:rows, off:off + w],
                                             start=(kt_i == 0), stop=(kt_i == CT - 1))
                        nc.vector.scalar_tensor_tensor(yt[:, off:off + w], ps[:, :w],
                                                       bfc2_t[:, m:m + 1], x_t[m][:, off:off + w],
                                                       op0=ALU.add, op1=ALU.add)
                    nc.sync.dma_start(yT[m * P:(m + 1) * P, :], yt[:])

    _split_multi_waits(nc)
    return nc


def _split_multi_waits(nc, max_waits=1):
    """This container's walrus encodes at most one sync-wait per instruction;
    hoist excess waits onto same-engine NoOps inserted just before."""
    counter = [0]
    for f in nc.m.functions:
        for bb in f.blocks:
            out = []
            changed = False
            for inst in bb.instructions:
                si = inst.sync_info
                waits = list(si.on_wait) if si is not None else []
                if len(waits) > max_waits:
                    changed = True
                    extra, keep = waits[:-max_waits], waits[-max_waits:]
                    for i in range(0, len(extra), max_waits):
                        nop = mybir.InstNoOp(name=f"I-waitsplit-{counter[0]}", ins=[], outs=[])
                        counter[0] += 1
                        nop.engine = inst.engine
                        nop.sync_info = mybir.SyncInfo(on_wait=extra[i:i + max_waits], on_update=[])
                        out.append(nop)
                    inst.sync_info = mybir.SyncInfo(on_wait=keep, on_update=list(si.on_update))
                out.append(inst)
            if changed:
                bb.instructions = out


def _prep_lhsT(w, n_mtiles):
    """[K, M] -> [n_mtiles, P, (K//P)*P]: tile[m][p, t*P+c] = w[t*P+p, m*P+c]."""
    K, M = w.shape
    out = np.zeros((n_mtiles, P, (K // P) * P), w.dtype)
    for m in range(n_mtiles):
        mc = w[:, m * P:min((m + 1) * P, M)]
        blk = mc.reshape(K // P, P, -1)
        for t in range(K // P):
            out[m, :, t * P:t * P + blk.shape[2]] = blk[t]
    return out


def prep_inputs(x, g1, w_qkv, b_qkv, w_proj, b_proj, g2, fc1_w, fc1_b, dw_w, dw_b, fc2_w, fc2_b):
    f32 = np.float32
    bf16 = ml_dtypes.bfloat16
    g1 = np.asarray(g1, f32); g2 = np.asarray(g2, f32)
    w_qkv = np.asarray(w_qkv, f32) * g1[:, None]
    b_qkv = np.asarray(b_qkv, f32).copy()
    w_qkv[:, :E] *= SCALE
    b_qkv[:E] *= SCALE

    wqk_a = _prep_lhsT(w_qkv[:, :2 * E], 16).astype(bf16)
    bqk_a = np.ascontiguousarray(b_qkv[:2 * E].reshape(16, P).T)
    wv_a = np.ascontiguousarray(w_qkv[:, 2 * E:]).astype(bf16)
    b_v = b_qkv[2 * E:]
    vaug = np.zeros((P, NH * (HD + 1)), f32)
    for h in range(NH):
        vaug[:, h * (HD + 1):h * (HD + 1) + HD] = b_v[h * HD:(h + 1) * HD][None, :]
        vaug[:, h * (HD + 1) + HD] = 1.0
    wproj_a = _prep_lhsT(np.asarray(w_proj, f32), ET).astype(bf16)
    bproj_a = np.ascontiguousarray(np.asarray(b_proj, f32).reshape(ET, P).T)

    # rope tables, feature-major (64 rows per head, replicated to 128)
    pos = np.arange(S - 1, dtype=f32)
    inv_freq = (10000.0 ** (-np.arange(0, HD, 2, dtype=f32) / HD))
    freqs = pos[:, None] * inv_freq[None, :]          # [1024, 32]
    cos_h = np.zeros((HD, S), f32); sin2_h = np.zeros((HD, S), f32)
    cos_h[:, 0] = 1.0
    c = np.cos(freqs).T; s = np.sin(freqs).T          # [32, 1024]
    cos_h[0:32, 1:] = c; cos_h[32:64, 1:] = c
    sin2_h[0:32, 1:] = s; sin2_h[32:64, 1:] = -s
    cosb_a = np.concatenate([cos_h, cos_h], 0)
    sin2b_a = np.concatenate([sin2_h, sin2_h], 0)

    fc1_w = np.asarray(fc1_w, f32) * g2[:, None]
    fc1_b = np.asarray(fc1_b, f32)
    wxg_a = _prep_lhsT(np.pad(fc1_w[:, :HID], ((0, 0), (0, CT * P - HID))), CT).astype(f32)
    wvg_a = _prep_lhsT(np.pad(fc1_w[:, HID:], ((0, 0), (0, CT * P - HID))), CT).astype(f32)
    bxg_a = np.pad(fc1_b[:HID], (0, CT * P - HID)).reshape(CT, P, 1).astype(f32)
    bvg_a = np.pad(fc1_b[HID:], (0, CT * P - HID)).reshape(CT, P, 1).astype(f32)
    dwt_a = np.asarray(dw_w, f32).reshape(9, HID).T   # [HID, 9], taps (dy,dx) row-major
    dwt_a = np.pad(dwt_a, ((0, CT * P - HID), (0, 0))).reshape(CT, P, 9)
    dwb_a = np.pad(np.asarray(dw_b, f32), (0, CT * P - HID)).reshape(CT, P, 1)

    fc2_pad = np.pad(np.asarray(fc2_w, f32), ((0, CT * P - HID), (0, 0)))   # [2816, 1024]
    wfc2_a = np.zeros((ET, P, CT * P), f32)
    for m in range(ET):
        wfc2_a[m] = fc2_pad[:, m * P:(m + 1) * P].reshape(CT, P, P).transpose(1, 0, 2).reshape(P, CT * P)
    bfc2_a = np.ascontiguousarray(np.asarray(fc2_b, f32).reshape(ET, P).T)

    shared = dict(
        wqk=wqk_a, bqk=bqk_a, wv=wv_a, vaug_init=vaug.astype(bf16),
        wproj=wproj_a, bproj=bproj_a, cosb=cosb_a, sin2b=sin2b_a,
        wxg=wxg_a, bxg=bxg_a, wvg=wvg_a, bvg=bvg_a, dwt=dwt_a, dwb=dwb_a,
        wfc2=wfc2_a.astype(bf16), bfc2=bfc2_a,
    )
    x = np.asarray(x, f32)
    return [dict(shared, xT=np.ascontiguousarray(x[b].T)) for b in range(B)]


_cached = {}


def kernel(**inputs) -> np.ndarray:
    vz = not np.any(np.asarray(inputs["b_qkv"])[2 * E:])
    if ("nc", vz) not in _cached:
        _cached[("nc", vz)] = build_kernel(VAUG_MEMSET=vz)
    nc = _cached[("nc", vz)]
    in_maps = prep_inputs(**{k: np.asarray(v) for k, v in inputs.items()})
    res = run_bass_kernel_spmd(nc, in_maps, list(range(NCORES)))
    out = np.stack([np.asarray(r["yT"]).T for r in res.results], axis=0)
    return np.ascontiguousarray(out).astype(np.float32)


if __name__ == "__main__":
    import reference
    inputs = {k: np.asarray(v) for k, v in reference.setup_inputs().items()}
    got = kernel(**inputs)
    exp = np.asarray(reference.reference(**inputs))
    err = np.abs(got - exp).max() / (np.abs(exp).max() + 1e-9)
    print("shapes", got.shape, exp.shape)
    print("max rel err vs absmax:", err)
